# revision 1
# baseline (speedup 1.0000x reference)
"""Trainium2 Bass kernel for nn_IterativeStructureRefiner.

Math restructuring (validated vs reference to ~1e-7):
  Per iteration (s = structure, cs = continuity*s):
    num = oxx.(hx + dpq/2) + oyy.(vy + dpq/2) + oxy.dmq
      where hx = csL+csR, vy/dpq/dmq are banded vertical stencils:
      vy = T0@cs, dpq = T0@hx, dmq = A0@(csL-csR)   (T0 = super+sub diag,
      A0 = super-sub diag) -> computed on the TensorEngine into PSUM:
        Sxx = (I + T0/2)@hx,   Syy = T0@cs + (T0/2)@hx,   dmq = A0@hd
    sm  = T9@(sL+sC+sR)      (T9 = tridiag(1/9): 3x3 box mean)
    s'  = 0.75*s + 0.25*g . (sm - s + r.num)
      with g = clip(1-clip(unc,0,1),0,1), r = 1/(den+1e-6) precomputed,
      den = same num-structure applied to continuity. The reference's final
      clip(.,0,1) is provably inactive (pre-clip in [0.5s, 0.5+0.5s]).

Sharding: pure data-parallel, one batch image per NeuronCore (B=8, 8 cores).
Each image is processed as 9x2 patches of [128 rows x 524 cols] with a 6-px
halo (interior 116x512): all 6 iterations run locally per patch -> zero
cross-patch/iteration HBM traffic; inputs are read once, output written once.
"""

import numpy as np

H = W = 1024
PATCH_W = 524          # patch-col space: image cols [c0-6, c0+518)
TILE_W = PATCH_W + 2   # +1 zero-pad col each side for shifted reads
ROWS_OUT = 116         # 128 partitions - 2*6 halo
NUM_ITERS = 6
EPS = 1e-6

_CACHE = {}


def _build_bass():
    import concourse.bacc as bacc
    import concourse.mybir as mybir
    from concourse.tile import TileContext

    fp32 = mybir.dt.float32
    Alu = mybir.AluOpType
    Act = mybir.ActivationFunctionType

    # Bacc (not raw Bass): its compile pass legalizes multi-sem waits, which
    # walrus codegen rejects ("Too many sync wait commands").
    nc = bacc.Bacc("TRN2", debug=False)

    cen_d = nc.dram_tensor("center", [H, W], fp32, kind="ExternalInput")
    con_d = nc.dram_tensor("continuity", [H, W], fp32, kind="ExternalInput")
    ori_d = nc.dram_tensor("orientation", [2, H, W], fp32, kind="ExternalInput")
    unc_d = nc.dram_tensor("uncertainty", [H, W], fp32, kind="ExternalInput")
    out_d = nc.dram_tensor("out", [H, W], fp32, kind="ExternalOutput")

    # Stationary matrices for the banded vertical stencils. matmul computes
    # lhsT.T @ rhs with contraction over partitions: out[m,:] = sum_k St[k,m]*in[k,:]
    k = np.arange(128)
    T0 = ((np.abs(k[:, None] - k[None, :]) == 1)).astype(np.float32)       # in[m-1]+in[m+1]
    Bm = np.eye(128, dtype=np.float32) + 0.5 * T0                          # in[m] + .5*(in[m-1]+in[m+1])
    H0 = 0.5 * T0
    A0 = ((k[:, None] == k[None, :] - 1).astype(np.float32)
          - (k[:, None] == k[None, :] + 1).astype(np.float32))             # in[m-1]-in[m+1]
    T9 = ((np.abs(k[:, None] - k[None, :]) <= 1)).astype(np.float32) / 9.0

    st_drams = [nc.inline_tensor(m, name=f"st_{i}")
                for i, m in enumerate([T0, Bm, H0, A0, T9])]
    # bottom row-panel valid-partition mask (p < 102): compute-engine APs
    # can't start at partition 102, so apply as per-partition tensor_scalar
    botmask_np = (np.arange(128) < 102).astype(np.float32)[:, None]
    bot_dram = nc.inline_tensor(botmask_np, name="botmask")

    row_panels = []
    for r0 in range(0, H, ROWS_OUT):
        r1 = min(r0 + ROWS_OUT, H)
        row_panels.append((r0, r1))
    col_panels = [0, 512]

    with TileContext(nc) as tc:
        with (
            tc.tile_pool(name="consts", bufs=1) as cpool,
            tc.tile_pool(name="inp", bufs=3) as ipool,
            tc.tile_pool(name="pre", bufs=2) as ppool,
            tc.tile_pool(name="scr", bufs=2) as spool,
            tc.tile_pool(name="psum", bufs=1, space="PSUM") as qpool,
        ):
            # stationaries -> SBUF once
            st = []
            for i, d in enumerate(st_drams):
                t = cpool.tile([128, 128], fp32, tag=f"st{i}")
                nc.sync.dma_start(out=t[:], in_=d[:, :])
                st.append(t)
            tT0, tB, tH0, tA0, tT9 = st
            botmask = cpool.tile([128, 1], fp32, tag="botmask")
            nc.sync.dma_start(out=botmask[:], in_=bot_dram[:, :])

            # persistent ping-pong structure tiles (edge cols zeroed once;
            # iteration writes cover [1:TILE_W-1] only)
            s_ab = []
            for nm in ("s_a", "s_b"):
                t = cpool.tile([128, TILE_W], fp32, tag=nm)
                nc.vector.memset(t[:, 0:1], 0.0)
                nc.vector.memset(t[:, TILE_W - 1:TILE_W], 0.0)
                s_ab.append(t)
            # persistent cs tile, same edge discipline
            cs = cpool.tile([128, TILE_W], fp32, tag="cs")
            nc.vector.memset(cs[:, 0:1], 0.0)
            nc.vector.memset(cs[:, TILE_W - 1:TILE_W], 0.0)

            for (r0, r1) in row_panels:
                for c0 in col_panels:
                    # ---- load inputs with halo; tile col t <-> image col c0-7+t
                    img_lo = max(c0 - 7, 0)
                    img_hi = min(c0 + 519, W)
                    t_lo = img_lo - (c0 - 7)
                    t_hi = img_hi - (c0 - 7)
                    row_lo = max(r0 - 6, 0)
                    row_hi = min(r0 + 122, H)
                    p_lo = row_lo - (r0 - 6)
                    p_hi = row_hi - (r0 - 6)

                    def load(src_ap, tag):
                        t = ipool.tile([128, TILE_W], fp32, tag=tag)
                        if t_lo > 0:
                            nc.gpsimd.memset(t[:, 0:t_lo], 0.0)
                        if t_hi < TILE_W:
                            nc.gpsimd.memset(t[:, t_hi:TILE_W], 0.0)
                        if p_lo > 0:
                            nc.gpsimd.memset(t[0:p_lo, t_lo:t_hi], 0.0)
                        if p_hi < 128:
                            # compute-engine APs need 32-aligned partition start;
                            # DMA below overwrites [aligned_lo:p_hi)
                            aligned_lo = (p_hi // 32) * 32
                            nc.gpsimd.memset(t[aligned_lo:128, t_lo:t_hi], 0.0)
                        nc.sync.dma_start(
                            out=t[p_lo:p_hi, t_lo:t_hi],
                            in_=src_ap[row_lo:row_hi, img_lo:img_hi])
                        return t

                    s0 = load(cen_d, "s0")
                    cont = load(con_d, "cont")
                    ox = load(ori_d[0], "ox")
                    oy = load(ori_d[1], "oy")
                    unc = load(unc_d, "unc")

                    # ---- per-patch precompute ----
                    oxx = ppool.tile([128, TILE_W], fp32, tag="oxx")
                    oyy = ppool.tile([128, TILE_W], fp32, tag="oyy")
                    oxy = ppool.tile([128, TILE_W], fp32, tag="oxy")
                    g4 = ppool.tile([128, TILE_W], fp32, tag="g4")
                    rmap = ppool.tile([128, PATCH_W], fp32, tag="rmap")
                    nc.scalar.activation(oxx[:], ox[:], Act.Square)
                    nc.scalar.activation(oyy[:], oy[:], Act.Square)
                    nc.gpsimd.tensor_mul(out=oxy[:], in0=ox[:], in1=oy[:])
                    c1 = spool.tile([128, TILE_W], fp32, tag="c1")
                    nc.vector.tensor_scalar(
                        out=c1[:], in0=unc[:], scalar1=1.0, scalar2=0.0,
                        op0=Alu.min, op1=Alu.max)
                    nc.vector.tensor_scalar(
                        out=g4[:], in0=c1[:], scalar1=-0.25, scalar2=0.25,
                        op0=Alu.mult, op1=Alu.add)
                    # Zero g4 on out-of-image pad regions: the update then
                    # leaves s=0 there every iteration, reproducing the
                    # reference's per-iteration zero padding at image edges.
                    if t_lo > 0:
                        nc.vector.memset(g4[:, 0:t_lo], 0.0)
                    if t_hi < TILE_W:
                        nc.vector.memset(g4[:, t_hi:TILE_W], 0.0)
                    if p_lo > 0:
                        nc.vector.memset(g4[0:p_lo, :], 0.0)
                    if p_hi < 128:
                        assert p_hi == 102
                        nc.vector.tensor_scalar(
                            out=g4[:], in0=g4[:], scalar1=botmask[:, 0:1],
                            scalar2=None, op0=Alu.mult)

                    IN = slice(1, 1 + PATCH_W)   # tile cols holding patch-col space

                    def vstencils(src_tile, hx_t, hd_t, sxx_q, syy_q, dmq_q):
                        """hx/hd from src (526-wide), then PE stencils into PSUM."""
                        nc.vector.tensor_add(out=hx_t[:], in0=src_tile[:, 0:PATCH_W],
                                             in1=src_tile[:, 2:TILE_W])
                        nc.gpsimd.tensor_sub(out=hd_t[:], in0=src_tile[:, 0:PATCH_W],
                                             in1=src_tile[:, 2:TILE_W])
                        for lo in (0, 512):
                            hi = min(lo + 512, PATCH_W)
                            nc.tensor.matmul(sxx_q[:, lo:hi], tB[:], hx_t[:, lo:hi],
                                             start=True, stop=True)
                            nc.tensor.matmul(syy_q[:, lo:hi], tT0[:],
                                             src_tile[:, 1 + lo:1 + hi], start=True, stop=False)
                            nc.tensor.matmul(syy_q[:, lo:hi], tH0[:], hx_t[:, lo:hi],
                                             start=False, stop=True)
                            nc.tensor.matmul(dmq_q[:, lo:hi], tA0[:], hd_t[:, lo:hi],
                                             start=True, stop=True)

                    def weighted_num(sxx_q, syy_q, dmq_q, out_t, tmp1, tmp2, tmp3):
                        nc.vector.tensor_mul(out=tmp1[:], in0=oxx[:, IN], in1=sxx_q[:])
                        nc.vector.tensor_mul(out=tmp2[:], in0=oyy[:, IN], in1=syy_q[:])
                        nc.vector.tensor_mul(out=tmp3[:], in0=oxy[:, IN], in1=dmq_q[:])
                        nc.vector.tensor_add(out=tmp1[:], in0=tmp1[:], in1=tmp2[:])
                        nc.gpsimd.tensor_add(out=out_t[:], in0=tmp1[:], in1=tmp3[:])

                    # den -> r
                    hxc = spool.tile([128, PATCH_W], fp32, tag="hx")
                    hdc = spool.tile([128, PATCH_W], fp32, tag="hd")
                    q_sxx = qpool.tile([128, PATCH_W], fp32, tag="q_sxx")
                    q_syy = qpool.tile([128, PATCH_W], fp32, tag="q_syy")
                    q_dmq = qpool.tile([128, PATCH_W], fp32, tag="q_dmq")
                    vstencils(cont, hxc, hdc, q_sxx, q_syy, q_dmq)
                    d1 = spool.tile([128, PATCH_W], fp32, tag="u1")
                    d2 = spool.tile([128, PATCH_W], fp32, tag="u2")
                    d3 = spool.tile([128, PATCH_W], fp32, tag="u3")
                    den = spool.tile([128, PATCH_W], fp32, tag="num")
                    weighted_num(q_sxx, q_syy, q_dmq, den, d1, d2, d3)
                    nc.vector.tensor_scalar_add(rmap[:], den[:], EPS)
                    nc.vector.reciprocal_approx_fast(out=rmap[:], in_=rmap[:])

                    # ---- 6 iterations ----
                    s_cur = s0
                    for it in range(NUM_ITERS):
                        s_nxt = s_ab[it % 2]
                        nc.gpsimd.tensor_mul(out=cs[:, IN], in0=cont[:, IN],
                                             in1=s_cur[:, IN])
                        hx = spool.tile([128, PATCH_W], fp32, tag="hx")
                        hd = spool.tile([128, PATCH_W], fp32, tag="hd")
                        q_sxx = qpool.tile([128, PATCH_W], fp32, tag="q_sxx")
                        q_syy = qpool.tile([128, PATCH_W], fp32, tag="q_syy")
                        q_dmq = qpool.tile([128, PATCH_W], fp32, tag="q_dmq")
                        vstencils(cs, hx, hd, q_sxx, q_syy, q_dmq)

                        # smooth: hs3 = sL+sC+sR ; sm = T9@hs3
                        hs3a = spool.tile([128, PATCH_W], fp32, tag="hs3a")
                        hs3 = spool.tile([128, PATCH_W], fp32, tag="hs3")
                        nc.vector.tensor_add(out=hs3a[:], in0=s_cur[:, 0:PATCH_W],
                                             in1=s_cur[:, 2:TILE_W])
                        nc.gpsimd.tensor_add(out=hs3[:], in0=hs3a[:],
                                             in1=s_cur[:, IN])
                        q_sm = qpool.tile([128, PATCH_W], fp32, tag="q_sm")
                        for lo in (0, 512):
                            hi = min(lo + 512, PATCH_W)
                            nc.tensor.matmul(q_sm[:, lo:hi], tT9[:], hs3[:, lo:hi],
                                             start=True, stop=True)

                        u1 = spool.tile([128, PATCH_W], fp32, tag="u1")
                        u2 = spool.tile([128, PATCH_W], fp32, tag="u2")
                        u3 = spool.tile([128, PATCH_W], fp32, tag="u3")
                        num = spool.tile([128, PATCH_W], fp32, tag="num")
                        weighted_num(q_sxx, q_syy, q_dmq, num, u1, u2, u3)

                        w1 = spool.tile([128, PATCH_W], fp32, tag="w1")
                        w2a = spool.tile([128, PATCH_W], fp32, tag="w2a")
                        w2 = spool.tile([128, PATCH_W], fp32, tag="w2")
                        uu = spool.tile([128, PATCH_W], fp32, tag="uu")
                        nc.vector.tensor_mul(out=w1[:], in0=rmap[:], in1=num[:])
                        nc.vector.tensor_sub(out=w2a[:], in0=q_sm[:], in1=s_cur[:, IN])
                        nc.gpsimd.tensor_add(out=w2[:], in0=w2a[:], in1=w1[:])
                        nc.vector.tensor_mul(out=uu[:], in0=g4[:, IN], in1=w2[:])
                        nc.vector.scalar_tensor_tensor(
                            out=s_nxt[:, IN], in0=s_cur[:, IN], scalar=0.75,
                            in1=uu[:], op0=Alu.mult, op1=Alu.add)
                        s_cur = s_nxt

                    # ---- store interior ----
                    nrows = r1 - r0
                    nc.sync.dma_start(
                        out=out_d[r0:r1, c0:c0 + 512],
                        in_=s_cur[6:6 + nrows, 7:519])

    nc.finalize()
    return nc


def kernel(center, continuity, orientation, uncertainty):
    from concourse.bass_utils import run_bass_kernel_spmd

    if "nc" not in _CACHE:
        _CACHE["nc"] = _build_bass()
    nc = _CACHE["nc"]

    B = center.shape[0]
    in_maps = []
    for b in range(B):
        in_maps.append({
            "center": np.ascontiguousarray(center[b, 0]),
            "continuity": np.ascontiguousarray(continuity[b, 0]),
            "orientation": np.ascontiguousarray(orientation[b]),
            "uncertainty": np.ascontiguousarray(uncertainty[b, 0]),
        })
    res = run_bass_kernel_spmd(nc, in_maps, core_ids=list(range(B)))
    out = np.stack([r["out"] for r in res.results])[:, None]
    return out.astype(np.float32)

